# revision 6
# baseline (speedup 1.0000x reference)
"""Cosine attention kernel for Trainium2, sharded over 8 NeuronCores.

Problem: N=4, L=S=2048, H=8, D=64 fp32.
  q = queries / ||queries||_D ; k = keys / ||keys||_D
  qk = einsum('nlhd,nshd->nlsh', q, k); A = softmax(qk / temp, axis=S)
  out = einsum('nlsh,nshd->nlhd', A, values)

Sharding: the 32 (n, h) pairs are split 4-per-core (data + head parallel).
Each core computes 4 independent 2048x2048 attention problems.

Per-core schedule (v4): ACT's softmax Exp over the full 2048x2048 score
matrix (128 x [128,1024] activations) is the critical resource; the PE is
close behind (bf16 matmuls, ~2.3GHz warm).  Keys are NOT pre-normalized:
K's row scale (rk = rsqrt(||k||^2)/temp) is folded into the Exp activation
as a per-partition `scale` operand (P^T partitions = s), so K's transposes
depend only on the DMA + a bf16 cast.  Q is normalized to bf16 on DVE
before its transposes (rq is per-l, the free dim, so it can't fold).  Per-
pair prep is software-pipelined under the previous pair's main loop; K's
square runs on the otherwise-idle GpSimd engine.

Main loop per (pair, 1024-col L-chunk):
  - P^T_raw[s_tile, l] = KcT_tile^T @ QnT into PSUM [128,1024] (2 bf16 mms)
  - ACT: pexp = Exp(P^T_raw * rk[s])  (PSUM -> SBUF bf16)
  - psum2[h][65, 512] += V_aug[s_tile]^T @ pexp_half (bf16, 16-step accum;
    row 64 accumulates the softmax denominator via V's ones column)
  - epilogue per 512-half: DVE copy -> PE transpose [65,128] -> [128,65] ->
    DVE reciprocal + scale -> one chunked DMA [128,4,64] per half.
"""

import sys

if "/opt/trn_rl_repo" not in sys.path:
    sys.path.insert(0, "/opt/trn_rl_repo")

import numpy as np

N_CORES = 8
PAIRS = 4          # (n, h) pairs per core
L = 2048           # query length
S = 2048           # key length
D = 64             # head dim
T = S // 128       # 128-row tiles per pair
LC = 2             # L chunks
LCHUNK = L // LC   # 1024

_PROGRAM_CACHE = {}


def _build_program():
    import concourse.tile as tile
    import concourse.bass as bass
    from concourse import bacc, mybir
    from concourse.bass import ds
    from concourse.masks import make_identity

    f32 = mybir.dt.float32
    bf16 = mybir.dt.bfloat16
    AF = mybir.ActivationFunctionType

    nc = bacc.Bacc("TRN2", target_bir_lowering=False, debug=False,
                   num_devices=N_CORES)
    q_hbm = nc.dram_tensor("q", [PAIRS, L, D], f32, kind="ExternalInput")
    k_hbm = nc.dram_tensor("k", [PAIRS, S, D], f32, kind="ExternalInput")
    v_hbm = nc.dram_tensor("v", [PAIRS, S, D], f32, kind="ExternalInput")
    t_hbm = nc.dram_tensor("temp", [1, 1], f32, kind="ExternalInput")
    o_hbm = nc.dram_tensor("o", [PAIRS, L, D], f32, kind="ExternalOutput")

    with tile.TileContext(nc) as tc:
        with (
            tc.tile_pool(name="const", bufs=1) as cpool,
            tc.tile_pool(name="raw", bufs=1) as raw_pool,
            tc.tile_pool(name="io", bufs=2) as io_pool,
            tc.tile_pool(name="work", bufs=2) as work_pool,
            tc.tile_pool(name="small", bufs=4) as small_pool,
            tc.tile_pool(name="pexp", bufs=6) as pexp_pool,
            tc.tile_pool(name="psum1", bufs=2, space="PSUM") as psum1_pool,
            tc.tile_pool(name="psum2", bufs=2, space="PSUM") as psum2_pool,
            tc.tile_pool(name="psmall", bufs=2, space="PSUM") as psmall_pool,
            tc.tile_pool(name="dram", bufs=1, space="DRAM") as dram_pool,
        ):
            identity = cpool.tile([128, 128], f32)
            make_identity(nc, identity[:])
            identity_bf = cpool.tile([128, 128], bf16)
            nc.vector.tensor_copy(identity_bf[:], identity[:])

            # Preload the Ln/Exp activation table set (natural_log_exp_and_
            # others) off the critical path: one tiny Ln now means no table
            # load between the first norm chain and the softmax Exps.
            tbl = cpool.tile([128, 1], f32)
            nc.vector.memset(tbl[:], 1.0)
            nc.scalar.activation(tbl[:], tbl[:], AF.Ln)

            # Warm-keeper ingredients: regular bf16 matmuls count as HAM
            # activity (transpose-mode does not), helping the PE p-state.
            scratch_f = cpool.tile([128, 512], f32)
            nc.vector.memset(scratch_f[:], 0.0)
            scratch_b = cpool.tile([128, 512], bf16)
            nc.vector.tensor_copy(scratch_b[:], scratch_f[:])
            scratch_w = cpool.tile([128, 128], bf16)
            nc.vector.memset(scratch_w[:], 0.0)

            def warm(n):
                for i in range(n):
                    wk = psmall_pool.tile([128, 512], f32, tag="tp", name="wk")
                    nc.tensor.matmul(wk[:], scratch_w[:], scratch_b[:])

            # HAM warmup while input DMAs stream in.
            warm(12)

            # 1/temp broadcast to [128, 1] (bounce through DRAM for the
            # partition-broadcast DMA).
            t_sb = cpool.tile([1, 1], f32)
            nc.sync.dma_start(t_sb[:], t_hbm.ap())
            rt_sb = cpool.tile([1, 1], f32)
            nc.vector.reciprocal(rt_sb[:], t_sb[:])
            rt_dram = dram_pool.tile([1, 1], f32)
            nc.sync.dma_start(rt_dram[:], rt_sb[:])
            rt_b = cpool.tile([128, 1], f32)
            nc.sync.dma_start(rt_b[:], rt_dram[:].to_broadcast([128, 1]))

            q_raw, k_raw = {}, {}
            v_stage, v_aug = {}, {}

            def dma_qk(p):
                # K first: its transposes only need the cast, so it
                # unblocks the PE earliest.
                k_raw[p] = raw_pool.tile([128, T, D], f32,
                                         tag=f"kraw{p}", name=f"kraw{p}")
                nc.sync.dma_start(
                    k_raw[p][:],
                    k_hbm.ap()[p].rearrange("(t pp) d -> pp t d", pp=128))
                q_raw[p] = raw_pool.tile([128, T, D], f32,
                                         tag=f"qraw{p}", name=f"qraw{p}")
                nc.sync.dma_start(
                    q_raw[p][:],
                    q_hbm.ap()[p].rearrange("(t pp) d -> pp t d", pp=128))

            def prep_v(p):
                # V with ones column appended; bf16 for mm2.
                v_stage[p] = io_pool.tile([128, T, D + 1], f32, tag="vstage",
                                          name=f"vstage{p}")
                nc.vector.memset(v_stage[p][:, :, D:D + 1], 1.0)
                nc.sync.dma_start(
                    v_stage[p][:, :, 0:D],
                    v_hbm.ap()[p].rearrange("(t pp) d -> pp t d", pp=128))
                v_aug[p] = io_pool.tile([128, T, D + 1], bf16, tag="vaug",
                                        name=f"vaug{p}")
                nc.vector.tensor_copy(v_aug[p][:], v_stage[p][:])

            dma_qk(0)
            prep_v(0)
            for p in range(1, PAIRS):
                dma_qk(p)

            rq, rk = {}, {}
            qnT, kcT = {}, {}

            def prep_k(p):
                # Unnormalized K: bf16 cast + transpose only.  rk is folded
                # into the softmax Exp's per-partition scale later.
                kc = work_pool.tile([128, T, D], bf16, tag="kn")
                nc.vector.tensor_copy(kc[:], k_raw[p][:])
                kcT[p] = raw_pool.tile([64, S], bf16, tag=f"kcT{p}",
                                       name=f"kcT{p}")
                for g in range(T // 4):
                    tp = psmall_pool.tile([64, 4, 128], bf16, tag="tp")
                    for j in range(4):
                        nc.tensor.transpose(
                            tp[:, j, :], kc[:, 4 * g + j, :], identity_bf[:])
                    nc.vector.tensor_copy(kcT[p][:, ds(512 * g, 512)], tp[:])
                # rk chain, off the critical path: square on GpSimd, reduce
                # on DVE, rsqrt via Ln+Exp (same ACT table set as softmax).
                sqk = work_pool.tile([128, T, D], f32, tag="sqk")
                nc.gpsimd.tensor_mul(sqk[:], k_raw[p][:], k_raw[p][:])
                ssk = cpool.tile([128, T], f32, name=f"ssk{p}", tag=f"ssk{p}")
                nc.vector.tensor_reduce(
                    ssk[:], sqk[:],
                    axis=mybir.AxisListType.X, op=mybir.AluOpType.add)
                rk[p] = cpool.tile([128, T], f32, name=f"rk{p}", tag=f"rk{p}")
                nc.scalar.activation(ssk[:], ssk[:], AF.Ln)
                nc.scalar.activation(rk[p][:], ssk[:], AF.Exp, scale=-0.5)
                nc.vector.tensor_scalar_mul(rk[p][:], rk[p][:], rt_b[:])

            def prep_q(p):
                # Q needs real normalization (rq is per-l, the free dim of
                # P^T, so it can't fold into the Exp scale).
                sqq = work_pool.tile([128, T, D], f32, tag="sqq")
                nc.vector.tensor_mul(sqq[:], q_raw[p][:], q_raw[p][:])
                ssq = cpool.tile([128, T], f32, name=f"ssq{p}", tag=f"ssq{p}")
                nc.vector.tensor_reduce(
                    ssq[:], sqq[:],
                    axis=mybir.AxisListType.X, op=mybir.AluOpType.add)
                rq_t = cpool.tile([128, T], f32, name=f"rq{p}", tag=f"rq{p}")
                nc.scalar.activation(ssq[:], ssq[:], AF.Ln)
                nc.scalar.activation(rq_t[:], ssq[:], AF.Exp, scale=-0.5)
                rq[p] = rq_t
                qn = work_pool.tile([128, T, D], bf16, tag="qn")
                r_b = bass.AP(tensor=rq_t[:].tensor, offset=rq_t[:].offset,
                              ap=[rq_t[:].ap[0], rq_t[:].ap[1], [0, D]])
                nc.vector.tensor_mul(qn[:], q_raw[p][:], r_b)
                qnT[p] = raw_pool.tile([64, L], bf16, tag=f"qnT{p}",
                                       name=f"qnT{p}")
                for g in range(T // 4):
                    tp = psmall_pool.tile([64, 4, 128], bf16, tag="tp")
                    for j in range(4):
                        nc.tensor.transpose(
                            tp[:, j, :], qn[:, 4 * g + j, :], identity_bf[:])
                    nc.vector.tensor_copy(qnT[p][:, ds(512 * g, 512)], tp[:])

            def main_chunk(p, lc):
                ps2 = {}
                for h in range(LCHUNK // 512):
                    ps2[h] = psum2_pool.tile([D + 1, 512], f32, tag="ps2",
                                             name=f"ps2_{h}")
                for st in range(T):
                    ps1 = psum1_pool.tile([128, LCHUNK], f32, tag="ps1")
                    lhs1 = kcT[p][:, ds(st * 128, 128)]
                    for h in range(LCHUNK // 512):
                        nc.tensor.matmul(
                            ps1[:, ds(h * 512, 512)], lhs1,
                            qnT[p][:, ds(lc * LCHUNK + h * 512, 512)])
                    pexp = pexp_pool.tile([128, LCHUNK], bf16, tag="pexp")
                    nc.scalar.activation(pexp[:], ps1[:], AF.Exp,
                                         scale=rk[p][:, st:st + 1])
                    lhs2 = v_aug[p][:, st, :]
                    for h in range(LCHUNK // 512):
                        nc.tensor.matmul(
                            ps2[h][:], lhs2,
                            pexp[:, ds(h * 512, 512)],
                            start=(st == 0), stop=(st == T - 1))

                # Epilogue per 512-half; one chunked DMA per half.
                for h in range(LCHUNK // 512):
                    o_sb = work_pool.tile([D + 1, 512], f32, tag="osb")
                    nc.vector.tensor_copy(o_sb[:], ps2[h][:])
                    o_fin = small_pool.tile([128, 4, D], f32, tag="ofin")
                    for j in range(512 // 128):
                        tp = psmall_pool.tile([128, D + 1], f32, tag="tp")
                        nc.tensor.transpose(
                            tp[:], o_sb[:, ds(j * 128, 128)],
                            identity[0:D + 1, 0:D + 1])
                        rcp = small_pool.tile([128, 1], f32, tag="rcp")
                        nc.vector.reciprocal(rcp[:], tp[:, D:D + 1])
                        nc.vector.tensor_scalar_mul(
                            o_fin[:, j, :], tp[:, 0:D], rcp[:])
                    nc.sync.dma_start(
                        o_hbm.ap()[p, ds(lc * LCHUNK + h * 512, 512), :]
                            .rearrange("(j pp) d -> pp j d", pp=128),
                        o_fin[:])

            # ---- Pipelined schedule: pair p+1's prep runs under pair p's
            # main loop; only pair 0's prep is exposed.
            prep_k(0)
            prep_q(0)
            prep_k(1)
            main_chunk(0, 0)
            prep_q(1)
            prep_v(1)
            main_chunk(0, 1)
            prep_k(2)
            main_chunk(1, 0)
            prep_q(2)
            prep_v(2)
            main_chunk(1, 1)
            prep_k(3)
            main_chunk(2, 0)
            prep_q(3)
            prep_v(3)
            main_chunk(2, 1)
            main_chunk(3, 0)
            main_chunk(3, 1)

    nc.compile()
    return nc


def _get_program():
    if "nc" not in _PROGRAM_CACHE:
        _PROGRAM_CACHE["nc"] = _build_program()
    return _PROGRAM_CACHE["nc"]


def kernel(queries, keys, values, temp_scale):
    from concourse.bass_utils import run_bass_kernel_spmd

    N, Lq, H, Dh = queries.shape
    assert (N, Lq, H, Dh) == (4, L, 8, D), (N, Lq, H, Dh)

    # [N, L, H, D] -> [N*H, L, D]; core c owns pairs 4c..4c+4.
    def shard(x):
        x = np.ascontiguousarray(
            np.asarray(x, dtype=np.float32).transpose(0, 2, 1, 3)
        ).reshape(N * H, Lq, Dh)
        return [np.ascontiguousarray(x[PAIRS * c:PAIRS * (c + 1)])
                for c in range(N_CORES)]

    qs, ks, vs = shard(queries), shard(keys), shard(values)
    t11 = np.asarray(temp_scale, dtype=np.float32).reshape(1, 1)
    in_maps = [
        {"q": qs[c], "k": ks[c], "v": vs[c], "temp": t11}
        for c in range(N_CORES)
    ]

    nc = _get_program()
    res = run_bass_kernel_spmd(nc, in_maps, core_ids=list(range(N_CORES)))
    if getattr(res, "exec_time_ns", None):
        print(f"HW exec time: {res.exec_time_ns} ns")

    out = np.stack([res.results[c]["o"] for c in range(N_CORES)])  # [8,4,L,D]
    out = out.reshape(N, H, Lq, Dh).transpose(0, 2, 1, 3)          # [N,L,H,D]
    return np.ascontiguousarray(out)


# revision 7
# speedup vs baseline: 1.0332x; 1.0332x over previous
"""Cosine attention kernel for Trainium2, sharded over 8 NeuronCores.

Problem: N=4, L=S=2048, H=8, D=64 fp32.
  q = queries / ||queries||_D ; k = keys / ||keys||_D
  qk = einsum('nlhd,nshd->nlsh', q, k); A = softmax(qk / temp, axis=2)
  out = einsum('nlsh,nshd->nlhd', A, values)

Sharding: the 32 (n, h) pairs are split 4-per-core (data + head parallel).
Each core computes 4 independent 2048x2048 attention problems.

Per-core schedule (v5): ACT's softmax Exp over the full 2048x2048 score
matrix is the critical resource, with the PE (bf16 matmuls, ~2.3GHz warm)
right behind.  Every 4th s-tile's exp is offloaded to the DVE using the
Schraudolph bit trick (exp(x) ~ reinterpret<int>(A*x + B), A = 2^23/ln2,
RMS rel err ~1.8% on [-1,1]; applied to 1/4 of elements -> ~0.9% output
L2, inside the 2e-2 gate), freeing ~25% of ACT time.  Per-pair prep (row
norms, rsqrt via one Ln+Exp ACT pair -- same act table set as the softmax
Exp, preloaded at start -- bf16 normalize, PE transposes, V load+cast) is
software-pipelined under the previous pair's main loop.

Main loop per (pair, 1024-col L-chunk):
  - P^T[s_tile, l] = KnT_tile^T @ QnT into PSUM [128,1024] (2 bf16 mms)
  - pexp = Exp(P^T) (ACT, or DVE bit-trick on every 4th s-tile) -> bf16
  - psum2[h][65, 512] += V_aug[s_tile]^T @ pexp_half (bf16, 16-step accum;
    row 64 accumulates the softmax denominator via V's ones column)
  - epilogue per 512-half: DVE copy -> PE transpose [65,128] -> [128,65] ->
    DVE reciprocal + scale -> one chunked DMA [128,4,64] per half.
"""

import sys

if "/opt/trn_rl_repo" not in sys.path:
    sys.path.insert(0, "/opt/trn_rl_repo")

import numpy as np

N_CORES = 8
PAIRS = 4          # (n, h) pairs per core
L = 2048           # query length
S = 2048           # key length
D = 64             # head dim
T = S // 128       # 128-row tiles per pair
LC = 2             # L chunks
LCHUNK = L // LC   # 1024

# Schraudolph fast-exp constants (numpy-validated: RMS rel err 1.77% on
# [-1,1] incl. bf16 output rounding, robust to round-vs-trunc int convert).
EXP_A = 12102203.161561  # 2^23 / ln 2
EXP_B = 1064876716.5     # 127<<23 minus RMS-optimal fudge

_PROGRAM_CACHE = {}


def _build_program():
    import concourse.tile as tile
    import concourse.bass as bass
    from concourse import bacc, mybir
    from concourse.bass import ds
    from concourse.masks import make_identity

    f32 = mybir.dt.float32
    bf16 = mybir.dt.bfloat16
    i32 = mybir.dt.int32
    AF = mybir.ActivationFunctionType

    nc = bacc.Bacc("TRN2", target_bir_lowering=False, debug=False,
                   num_devices=N_CORES)
    q_hbm = nc.dram_tensor("q", [PAIRS, L, D], f32, kind="ExternalInput")
    k_hbm = nc.dram_tensor("k", [PAIRS, S, D], f32, kind="ExternalInput")
    v_hbm = nc.dram_tensor("v", [PAIRS, S, D], f32, kind="ExternalInput")
    t_hbm = nc.dram_tensor("temp", [1, 1], f32, kind="ExternalInput")
    o_hbm = nc.dram_tensor("o", [PAIRS, L, D], f32, kind="ExternalOutput")

    with tile.TileContext(nc) as tc:
        with (
            tc.tile_pool(name="const", bufs=1) as cpool,
            tc.tile_pool(name="raw", bufs=1) as raw_pool,
            tc.tile_pool(name="io", bufs=2) as io_pool,
            tc.tile_pool(name="work", bufs=2) as work_pool,
            tc.tile_pool(name="small", bufs=4) as small_pool,
            tc.tile_pool(name="pexp", bufs=6) as pexp_pool,
            tc.tile_pool(name="psum1", bufs=2, space="PSUM") as psum1_pool,
            tc.tile_pool(name="psum2", bufs=2, space="PSUM") as psum2_pool,
            tc.tile_pool(name="psmall", bufs=2, space="PSUM") as psmall_pool,
            tc.tile_pool(name="dram", bufs=1, space="DRAM") as dram_pool,
        ):
            identity = cpool.tile([128, 128], f32)
            make_identity(nc, identity[:])
            identity_bf = cpool.tile([128, 128], bf16)
            nc.vector.tensor_copy(identity_bf[:], identity[:])

            # Preload the Ln/Exp activation table set off the critical path:
            # no table load between the first norm chain and the softmax Exp.
            tbl = cpool.tile([128, 1], f32)
            nc.vector.memset(tbl[:], 1.0)
            nc.scalar.activation(tbl[:], tbl[:], AF.Ln)

            # Warm-keeper ingredients: regular bf16 matmuls count as HAM
            # activity (transpose-mode does not), keeping the PE p-state up.
            scratch_f = cpool.tile([128, 512], f32)
            nc.vector.memset(scratch_f[:], 0.0)
            scratch_b = cpool.tile([128, 512], bf16)
            nc.vector.tensor_copy(scratch_b[:], scratch_f[:])
            scratch_w = cpool.tile([128, 128], bf16)
            nc.vector.memset(scratch_w[:], 0.0)

            def warm(n):
                # fresh pool tiles each time: never pins a psmall slot
                for i in range(n):
                    wk = psmall_pool.tile([128, 512], f32, tag="tp", name="wk")
                    nc.tensor.matmul(wk[:], scratch_w[:], scratch_b[:])

            # HAM warmup while input DMAs stream in.
            warm(12)

            # 1/temp broadcast to [128, 1] (bounce through DRAM for the
            # partition-broadcast DMA).
            t_sb = cpool.tile([1, 1], f32)
            nc.sync.dma_start(t_sb[:], t_hbm.ap())
            rt_sb = cpool.tile([1, 1], f32)
            nc.vector.reciprocal(rt_sb[:], t_sb[:])
            rt_dram = dram_pool.tile([1, 1], f32)
            nc.sync.dma_start(rt_dram[:], rt_sb[:])
            rt_b = cpool.tile([128, 1], f32)
            nc.sync.dma_start(rt_b[:], rt_dram[:].to_broadcast([128, 1]))

            q_raw, k_raw = {}, {}
            v_stage, v_aug = {}, {}

            def dma_qk(p):
                q_raw[p] = raw_pool.tile([128, T, D], f32,
                                         tag=f"qraw{p}", name=f"qraw{p}")
                nc.sync.dma_start(
                    q_raw[p][:],
                    q_hbm.ap()[p].rearrange("(t pp) d -> pp t d", pp=128))
                k_raw[p] = raw_pool.tile([128, T, D], f32,
                                         tag=f"kraw{p}", name=f"kraw{p}")
                nc.sync.dma_start(
                    k_raw[p][:],
                    k_hbm.ap()[p].rearrange("(t pp) d -> pp t d", pp=128))

            def prep_v(p):
                # V with ones column appended; bf16 for mm2.
                v_stage[p] = io_pool.tile([128, T, D + 1], f32, tag="vstage",
                                          name=f"vstage{p}")
                nc.vector.memset(v_stage[p][:, :, D:D + 1], 1.0)
                nc.sync.dma_start(
                    v_stage[p][:, :, 0:D],
                    v_hbm.ap()[p].rearrange("(t pp) d -> pp t d", pp=128))
                v_aug[p] = io_pool.tile([128, T, D + 1], bf16, tag="vaug",
                                        name=f"vaug{p}")
                nc.vector.tensor_copy(v_aug[p][:], v_stage[p][:])

            dma_qk(0)
            prep_v(0)
            for p in range(1, PAIRS):
                dma_qk(p)

            ssq, r_n, rq, rk = {}, {}, {}, {}

            def prep_norm(p):
                # Row sum-of-squares on DVE; rsqrt as exp(-0.5*ln(ssq)).
                ssq[p] = cpool.tile([128, 2, T], f32, name=f"ssq{p}",
                                    tag=f"ssq{p}")
                for i, srct in ((0, q_raw[p]), (1, k_raw[p])):
                    sq = work_pool.tile([128, T, D], f32, tag="sq")
                    nc.vector.tensor_mul(sq[:], srct[:], srct[:])
                    nc.vector.tensor_reduce(
                        ssq[p][:, i, :], sq[:],
                        axis=mybir.AxisListType.X, op=mybir.AluOpType.add)
                r_n[p] = cpool.tile([128, 2, T], f32, name=f"r_n{p}",
                                    tag=f"r_n{p}")
                nc.scalar.activation(ssq[p][:], ssq[p][:], AF.Ln)
                nc.scalar.activation(r_n[p][:], ssq[p][:], AF.Exp, scale=-0.5)
                rq[p] = r_n[p][:, 0, :]
                rk[p] = r_n[p][:, 1, :]
                nc.vector.tensor_scalar_mul(rk[p], rk[p], rt_b[:])

            qnT, knT = {}, {}

            def prep_tp(p):
                # Normalize to bf16 + PE-transpose to [64, 2048].
                qn = work_pool.tile([128, T, D], bf16, tag="qn")
                kn = work_pool.tile([128, T, D], bf16, tag="kn")
                for rr, srct, dstt in ((rq[p], q_raw[p], qn),
                                       (rk[p], k_raw[p], kn)):
                    r_b = bass.AP(tensor=rr.tensor, offset=rr.offset,
                                  ap=[rr.ap[0], rr.ap[1], [0, D]])
                    nc.vector.tensor_mul(dstt[:], srct[:], r_b)
                qnT[p] = raw_pool.tile([64, L], bf16, tag=f"qnT{p}",
                                       name=f"qnT{p}")
                knT[p] = raw_pool.tile([64, S], bf16, tag=f"knT{p}",
                                       name=f"knT{p}")
                for srct, dstt in ((qn, qnT[p]), (kn, knT[p])):
                    for g in range(T // 4):
                        tp = psmall_pool.tile([64, 4, 128], bf16, tag="tp")
                        for j in range(4):
                            nc.tensor.transpose(
                                tp[:, j, :], srct[:, 4 * g + j, :],
                                identity_bf[:])
                        nc.vector.tensor_copy(dstt[:, ds(512 * g, 512)], tp[:])
                    warm(1)  # keep the HAM busy-window alive

            def main_chunk(p, lc):
                ps2 = {}
                for h in range(LCHUNK // 512):
                    ps2[h] = psum2_pool.tile([D + 1, 512], f32, tag="ps2",
                                             name=f"ps2_{h}")
                for st in range(T):
                    ps1 = psum1_pool.tile([128, LCHUNK], f32, tag="ps1")
                    lhs1 = knT[p][:, ds(st * 128, 128)]
                    for h in range(LCHUNK // 512):
                        nc.tensor.matmul(
                            ps1[:, ds(h * 512, 512)], lhs1,
                            qnT[p][:, ds(lc * LCHUNK + h * 512, 512)])
                    pexp = pexp_pool.tile([128, LCHUNK], bf16, tag="pexp")
                    if st % 4 == 3:
                        # DVE fast-exp: int(A*x+B) bit-reinterpreted as f32.
                        it = work_pool.tile([128, LCHUNK], i32, tag="sch")
                        nc.vector.tensor_scalar(
                            it[:], ps1[:], EXP_A, EXP_B,
                            mybir.AluOpType.mult, mybir.AluOpType.add)
                        nc.vector.tensor_copy(pexp[:], it[:].bitcast(f32))
                    else:
                        nc.scalar.activation(pexp[:], ps1[:], AF.Exp)
                    lhs2 = v_aug[p][:, st, :]
                    for h in range(LCHUNK // 512):
                        nc.tensor.matmul(
                            ps2[h][:], lhs2,
                            pexp[:, ds(h * 512, 512)],
                            start=(st == 0), stop=(st == T - 1))

                # Epilogue per 512-half; one chunked DMA per half.
                for h in range(LCHUNK // 512):
                    o_sb = work_pool.tile([D + 1, 512], f32, tag="osb")
                    nc.vector.tensor_copy(o_sb[:], ps2[h][:])
                    o_fin = small_pool.tile([128, 4, D], f32, tag="ofin")
                    for j in range(512 // 128):
                        tp = psmall_pool.tile([128, D + 1], f32, tag="tp")
                        nc.tensor.transpose(
                            tp[:], o_sb[:, ds(j * 128, 128)],
                            identity[0:D + 1, 0:D + 1])
                        rcp = small_pool.tile([128, 1], f32, tag="rcp")
                        nc.vector.reciprocal(rcp[:], tp[:, D:D + 1])
                        nc.vector.tensor_scalar_mul(
                            o_fin[:, j, :], tp[:, 0:D], rcp[:])
                    nc.sync.dma_start(
                        o_hbm.ap()[p, ds(lc * LCHUNK + h * 512, 512), :]
                            .rearrange("(j pp) d -> pp j d", pp=128),
                        o_fin[:])

            # ---- Pipelined schedule: pair p+1's prep runs under pair p's
            # main loop; only pair 0's prep is exposed.
            prep_norm(0)
            prep_tp(0)
            prep_norm(1)
            main_chunk(0, 0)
            prep_tp(1)
            prep_v(1)
            main_chunk(0, 1)
            prep_norm(2)
            main_chunk(1, 0)
            prep_tp(2)
            prep_v(2)
            main_chunk(1, 1)
            prep_norm(3)
            main_chunk(2, 0)
            prep_tp(3)
            prep_v(3)
            main_chunk(2, 1)
            main_chunk(3, 0)
            main_chunk(3, 1)

    nc.compile()
    return nc


def _get_program():
    if "nc" not in _PROGRAM_CACHE:
        _PROGRAM_CACHE["nc"] = _build_program()
    return _PROGRAM_CACHE["nc"]


def kernel(queries, keys, values, temp_scale):
    from concourse.bass_utils import run_bass_kernel_spmd

    N, Lq, H, Dh = queries.shape
    assert (N, Lq, H, Dh) == (4, L, 8, D), (N, Lq, H, Dh)

    # [N, L, H, D] -> [N*H, L, D]; core c owns pairs 4c..4c+4.
    def shard(x):
        x = np.ascontiguousarray(
            np.asarray(x, dtype=np.float32).transpose(0, 2, 1, 3)
        ).reshape(N * H, Lq, Dh)
        return [np.ascontiguousarray(x[PAIRS * c:PAIRS * (c + 1)])
                for c in range(N_CORES)]

    qs, ks, vs = shard(queries), shard(keys), shard(values)
    t11 = np.asarray(temp_scale, dtype=np.float32).reshape(1, 1)
    in_maps = [
        {"q": qs[c], "k": ks[c], "v": vs[c], "temp": t11}
        for c in range(N_CORES)
    ]

    nc = _get_program()
    res = run_bass_kernel_spmd(nc, in_maps, core_ids=list(range(N_CORES)))
    if getattr(res, "exec_time_ns", None):
        print(f"HW exec time: {res.exec_time_ns} ns")

    out = np.stack([res.results[c]["o"] for c in range(N_CORES)])  # [8,4,L,D]
    out = out.reshape(N, H, Lq, Dh).transpose(0, 2, 1, 3)          # [N,L,H,D]
    return np.ascontiguousarray(out)
